# revision 8
# baseline (speedup 1.0000x reference)
"""
Trainium2 kernel for nn_CanonicalLinear (dense_mlp).

Math: out = x @ W_eff.T + b_eff with W_eff = sum_n f[n] W[n],
b_eff = sum_n f[n] b[n] (8x FLOP reduction vs per-head form).

Sharding: DP=2 (batch) x TP=4 (classes); core r=(p,q) computes
out[p-half, q-quarter].  Host supplies x as [d, bs] bf16 and W as
[n, d, csi] bf16 (the core's HALF of its class quarter; the other
half comes from the batch-peer via per-group AllGather of the
factor-REDUCED chunks, which are 8x smaller than raw W).

k-MAJOR emission: for each 128-row d-chunk k, the program emits (in
this order) the W-chunk DMA + its 3-engine reduce, the x-row DMAs
for the window blocks, the window-wave matmuls for chunk k, and any
pending PSUM evictions — so every engine queue sees work in the same
order dependencies resolve, and nothing queues behind the whole W
phase.

Window schedule (8 PSUM banks, bias folded in as a rank-1 K=1
matmul so evictions are plain copies):
  k=0..7:  tiles 0-7 accumulate half-chains (paced by arrivals)
  k=8..15: per chunk, two tile-(0-7) partials evict (DVE/Act) and
           two tiles of 8-15 burst their k0-7 half-chains (chunks
           already present -> full PE speed)
  post:    tiles 0-7 then 8-15 second halves (k8-15, full speed),
           final evict adds the partial (DVE) and stores bf16;
           tiles 16-31 run sequential full chains with Act-copy
           evictions.
"""

import os

import numpy as np

P = 128
B, D, C, N = 8192, 2048, 2048, 8
DP, TP = 2, 4
BS, CS = B // DP, C // TP          # 4096, 512
CSI = CS // DP                     # 256: per-core W slice width
NCORES = 8
DK = D // P                        # 16
NBT = BS // P                      # 32
BLK = 4                            # b-tiles per x DMA block
KH = DK // 2                       # 8: k-split point
GE = 4                             # chunks per AllGather
GROUPS = [[q + i * TP for i in range(DP)] for q in range(TP)]
W_SPLIT = False                    # collectives cost ~15us fixed: not worth it

_cached_nc = None


def _build(bs=BS, cs=CS, repeat=1, w_split=None):
    if w_split is None:
        w_split = W_SPLIT
    import concourse.bass as bass
    import concourse.mybir as mybir
    import concourse.tile as tile
    from concourse import bacc

    FP32 = mybir.dt.float32
    BF16 = mybir.dt.bfloat16
    MULT = mybir.AluOpType.mult
    ADD = mybir.AluOpType.add
    ACT_COPY = mybir.ActivationFunctionType.Copy

    nbt = bs // P
    csi = cs // DP if w_split else cs

    nc = bacc.Bacc()
    xd = nc.dram_tensor("x", [D, bs], BF16, kind="ExternalInput")
    wd = nc.dram_tensor("w", [N, D, csi], BF16, kind="ExternalInput")
    bd = nc.dram_tensor("b", [N, cs], FP32, kind="ExternalInput")
    fd = nc.dram_tensor("f", [N], FP32, kind="ExternalInput")
    od = nc.dram_tensor("out", [bs, cs], BF16, kind="ExternalOutput")
    if w_split:
        whalf = nc.dram_tensor("whalf", [D, csi], BF16)
        ngath = DK // GE
        wgathers = [nc.dram_tensor(f"wgather{g}", [DP * GE * P, csi], BF16)
                    for g in range(ngath)]

    with tile.TileContext(nc) as tc:
        with (
            tc.tile_pool(name="singles", bufs=1) as singles,
            tc.tile_pool(name="wload", bufs=3) as wload,
            tc.tile_pool(name="scp", bufs=16) as scp,
            tc.tile_pool(name="acp", bufs=12) as acp,
            tc.tile_pool(name="wkp", bufs=DK) as wkp,
            tc.tile_pool(name="xtp", bufs=6) as xtp,
            tc.tile_pool(name="pab", bufs=16) as pab,
            tc.tile_pool(name="outp", bufs=4) as outp,
            tc.tile_pool(name="pso", bufs=8, space="PSUM") as pso,
        ):
            # --- factor broadcast + per-engine copies -------------------
            f_ap = fd[:]
            f_rep = singles.tile([P, N], FP32)
            nc.gpsimd.dma_start(
                f_rep,
                bass.AP(tensor=f_ap.tensor, offset=f_ap.offset,
                        ap=[[0, P]] + list(f_ap.ap)),
            )
            f_use = singles.tile([P, N], FP32)
            nc.vector.tensor_copy(f_use, f_rep)
            f_use3 = singles.tile([P, N], FP32)
            nc.scalar.copy(f_use3, f_rep)

            # --- b_eff row (K=8 matmul), bf16, + ones column ------------
            b_sb = singles.tile([N, cs], FP32)
            nc.sync.dma_start(b_sb, bd[:])
            f8 = singles.tile([N, 1], FP32)
            nc.sync.dma_start(
                f8,
                bass.AP(tensor=f_ap.tensor, offset=f_ap.offset,
                        ap=list(f_ap.ap) + [[1, 1]]),
            )
            beff_row = singles.tile([1, cs], FP32)
            pw = pso.tile([1, 512], FP32, name="po", tag="po")
            nc.tensor.matmul(pw[:, :cs], f8, b_sb)
            nc.any.tensor_copy(beff_row, pw[:, :cs])
            beff16 = singles.tile([1, cs], BF16)
            nc.vector.tensor_copy(beff16, beff_row)
            ones1b = singles.tile([1, P], BF16)
            nc.vector.memset(ones1b, 1.0)

            for _rep in range(repeat):
                wk_tiles = [None] * DK
                xtb_blocks = {}

                def get_block_tile(blk, xtb_blocks=xtb_blocks):
                    if blk not in xtb_blocks and blk * BLK < nbt:
                        xtb_blocks[blk] = (
                            xtp.tile([P, DK, BLK * P], BF16, name="xtb"),
                            set())
                    return xtb_blocks.get(blk)

                def emit_xblock(blk):
                    # single 3D-AP DMA for a whole [P, DK, BLK*P] block
                    ent = get_block_tile(blk)
                    if ent is None or len(ent[1]) == DK:
                        return
                    xtb, done = ent
                    assert not done, "block partially row-loaded"
                    done.update(range(DK))
                    nt = min(BLK, nbt - blk * BLK)
                    x_ap = xd[:]
                    nc.scalar.dma_start(
                        xtb[:, :, :nt * P],
                        bass.AP(tensor=x_ap.tensor,
                                offset=x_ap.offset + blk * BLK * P,
                                ap=[[bs, P], [P * bs, DK], [1, nt * P]]),
                    )

                def emit_xrow(blk, k):
                    ent = get_block_tile(blk)
                    if ent is None or k in ent[1]:
                        return
                    xtb, done = ent
                    done.add(k)
                    nt = min(BLK, nbt - blk * BLK)
                    nc.scalar.dma_start(
                        xtb[:, k, :nt * P],
                        xd[k * P:(k + 1) * P,
                           blk * BLK * P:blk * BLK * P + nt * P])

                wdma_done = {}

                def emit_wdma(k):
                    if k in wdma_done:
                        return wdma_done[k]
                    wblk = wload.tile([P, N, csi], BF16)
                    w_ap = wd[:]
                    # two 4-head halves: the reduce on heads 0-3 starts
                    # ~1.5us before the full chunk lands
                    for h in range(2):
                        nc.sync.dma_start(
                            wblk[:, 4 * h:4 * h + 4, :],
                            bass.AP(tensor=w_ap.tensor,
                                    offset=(w_ap.offset + k * P * csi
                                            + 4 * h * D * csi),
                                    ap=[[csi, P], [D * csi, 4], [1, csi]]),
                        )
                    wdma_done[k] = wblk
                    return wblk

                def emit_wchunk(k):
                    wblk = emit_wdma(k)
                    s = {}
                    for n in (0, 1, 2, 3, 7):      # DVE scalings (4x)
                        s[n] = scp.tile([P, csi], BF16, name="s")
                        nc.vector.tensor_scalar(
                            s[n], wblk[:, n, :], f_use[:, n:n + 1],
                            None, MULT)
                    for n in (4, 5, 6):            # Activation scalings
                        s[n] = scp.tile([P, csi], BF16, name="s")
                        nc.scalar.activation(
                            s[n], wblk[:, n, :], ACT_COPY,
                            scale=f_use3[:, n:n + 1])
                    a01 = acp.tile([P, csi], BF16, name="a")
                    nc.vector.tensor_tensor(a01, s[0], s[1], ADD)
                    a23 = acp.tile([P, csi], BF16, name="a")
                    nc.vector.tensor_tensor(a23, s[2], s[3], ADD)
                    aa = acp.tile([P, csi], BF16, name="a")
                    nc.vector.tensor_tensor(aa, a01, a23, ADD)
                    aa7 = acp.tile([P, csi], BF16, name="a")
                    nc.vector.tensor_tensor(aa7, aa, s[7], ADD)
                    a45 = acp.tile([P, csi], BF16, name="a")
                    nc.gpsimd.tensor_tensor(a45, s[4], s[5], ADD)
                    a456 = acp.tile([P, csi], BF16, name="a")
                    nc.vector.tensor_tensor(a456, a45, s[6], ADD)
                    if w_split:
                        wh = acp.tile([P, csi], BF16, name="a")
                        nc.gpsimd.tensor_tensor(wh, aa7, a456, ADD)
                        nc.sync.dma_start(whalf[k * P:(k + 1) * P, :], wh)
                        if k % GE == GE - 1:
                            g = k // GE
                            nc.gpsimd.collective_compute(
                                "AllGather",
                                mybir.AluOpType.bypass,
                                replica_groups=GROUPS,
                                ins=[whalf[g * GE * P:(g + 1) * GE * P, :]],
                                outs=[wgathers[g][:]],
                            )
                            for j in range(GE):
                                kk = g * GE + j
                                wk = wkp.tile([P, cs], BF16, name="wk")
                                for m in range(DP):
                                    nc.scalar.dma_start(
                                        wk[:, m * csi:(m + 1) * csi],
                                        wgathers[g][(m * GE + j) * P:
                                                    (m * GE + j + 1) * P, :])
                                wk_tiles[kk] = wk
                    else:
                        wk = wkp.tile([P, cs], BF16, name="wk")
                        nc.gpsimd.tensor_tensor(wk, aa7, a456, ADD)
                        wk_tiles[k] = wk

                def bias_mm(po):
                    nc.tensor.matmul(po[:, :], ones1b, beff16[:1, :],
                                     start=True, stop=False)

                def mm(po, i, k, stop):
                    xtb, _ = get_block_tile(i // BLK)
                    u = i % BLK
                    nc.tensor.matmul(
                        po[:, :],
                        xtb[:, k, u * P:(u + 1) * P],
                        wk_tiles[k][:, :],
                        start=False, stop=stop,
                    )

                # ---- window: tiles 0-7 as full-k sessions --------------
                # 8 PSUM banks <=> 8 sessions; each chunk k feeds 8 matmuls
                # as it arrives.  Only x blocks 0-1 (the window tiles) are
                # streamed during the W window; later blocks follow.
                pcur = {}

                def final_evict(i, po):
                    osb = outp.tile([P, cs], BF16)
                    if i % 2 == 0:
                        nc.vector.tensor_copy(osb, po)
                    else:
                        nc.scalar.copy(osb, po)
                    nc.sync.dma_start(od[i * P:(i + 1) * P, :], osb)

                for k in range(3):
                    emit_wdma(k)
                    emit_xrow(0, k)
                    emit_xrow(1, k)
                for k in range(DK):
                    emit_wchunk(k)
                    emit_xrow(0, k)
                    emit_xrow(1, k)
                    if k == 0:
                        for i in range(8):
                            pcur[i] = pso.tile([P, cs], FP32, name="po",
                                               tag="po")
                            bias_mm(pcur[i])
                    for i in range(8):
                        mm(pcur[i], i, k, stop=(k == DK - 1))

                # next x block streams while the window tiles drain
                for k in range(DK):
                    emit_xrow(2, k)
                for i in range(8):
                    final_evict(i, pcur[i])

                # sequential full chains for tiles 8..nbt-1
                for i in range(8, nbt):
                    blk = i // BLK
                    if i % BLK == 0:
                        for k in range(DK):
                            emit_xrow(blk, k)
                            emit_xrow(blk + 1, k)
                    po = pso.tile([P, cs], FP32, name="po", tag="po")
                    bias_mm(po)
                    for k in range(DK):
                        mm(po, i, k, stop=(k == DK - 1))
                    final_evict(i, po)

    nc.finalize()
    return nc


def _build_repeat(repeat):
    return _build(repeat=repeat)


def _get_nc():
    global _cached_nc
    if _cached_nc is None:
        _cached_nc = _build(repeat=int(os.environ.get("KREPEAT", "1")))
    return _cached_nc


def _shard_inputs(x, W, b, factor):
    import ml_dtypes
    BF = ml_dtypes.bfloat16

    xT = np.ascontiguousarray(x.T.astype(BF))                   # [D, B]
    Wt = np.ascontiguousarray(W.transpose(0, 2, 1).astype(BF))  # [N, D, C]
    in_maps = []
    for r in range(NCORES):
        p, q = divmod(r, TP)
        in_maps.append({
            "x": np.ascontiguousarray(xT[:, p * BS:(p + 1) * BS]),
            "w": np.ascontiguousarray(
                Wt[:, :, q * CS + p * CSI:q * CS + (p + 1) * CSI]
                if W_SPLIT else Wt[:, :, q * CS:(q + 1) * CS]),
            "b": np.ascontiguousarray(b[:, q * CS:(q + 1) * CS]),
            "f": np.ascontiguousarray(factor),
        })
    return in_maps


def _unshard_into(out, r, oc):
    p, q = divmod(r, TP)
    out[p * BS:(p + 1) * BS, q * CS:(q + 1) * CS] = \
        np.asarray(oc, dtype=np.float32)


def kernel(x, W, b, factor, _trace=False):
    from concourse.bass_utils import run_bass_kernel_spmd

    x = np.asarray(x, dtype=np.float32)
    W = np.asarray(W, dtype=np.float32)
    b = np.asarray(b, dtype=np.float32)
    factor = np.asarray(factor, dtype=np.float32)

    nc = _get_nc()
    in_maps = _shard_inputs(x, W, b, factor)
    res = run_bass_kernel_spmd(nc, in_maps, list(range(NCORES)),
                               trace=_trace)

    out = np.empty((B, C), dtype=np.float32)
    for r in range(NCORES):
        _unshard_into(out, r, res.results[r]["out"])
    if _trace:
        return out, res
    return out
